# revision 1
# baseline (speedup 1.0000x reference)
"""LightGCN 2-hop smoothing on 8 Trainium2 NeuronCores.

Strategy (edge-sharded by destination):
  - Host: build symmetric directed edge list (2E = 2.5M messages), compute
    symmetric GCN weights w_e from degrees (index-only metadata), sort edges
    by destination, and pack them into fixed-size 128-edge chunks grouped by
    128-node destination blocks. Core c owns destination nodes
    [c*25088, (c+1)*25088).
  - Device, per smoothing hop: gather source rows from a replicated bf16
    node-embedding table with indirect DMA (128*G rows per instruction),
    build a weighted one-hot selection matrix per 128-edge chunk on the DVE
    (out[p,f] = (f == dstloc[p]) * w[p]), and matmul-accumulate the chunk's
    messages into a PSUM tile per destination block:
        psum[slot, :] += sum_e w_e * table[src_e, :]
  - Between hops: AllGather the bf16 x1 shards into a full replicated table.
  - Final output out = (2*x0 + 2*x1 + x2)/3 assembled at hop-2 eviction from
    an SBUF-resident fp32 copy of x1 plus the fp32 x0 shard.
"""

import numpy as np

import concourse.bass as bass
import concourse.bacc as bacc
import concourse.mybir as mybir
import concourse.tile as tile
from concourse.bass import IndirectOffsetOnAxis
from concourse.bass_utils import run_bass_kernel_spmd

NU = 100000          # num users
NI = 100000          # num items
N = NU + NI          # real nodes
D = 64               # embedding dim
NCORES = 8
R = 25088            # padded rows per core (196 blocks of 128)
NPAD = R * NCORES    # 200704 padded node table rows
NB = 196             # destination blocks per core
GB = 4               # blocks per gather group
NG = NB // GB        # gather groups per core

F32 = mybir.dt.float32
BF16 = mybir.dt.bfloat16
I32 = mybir.dt.int32
NP_BF16 = mybir.dt.np(mybir.dt.bfloat16)

_PROG_CACHE = {}


def _host_prep(u_emb, i_emb, u_idx, i_idx):
    i_g = i_idx.astype(np.int64) + NU
    src = np.concatenate([u_idx.astype(np.int64), i_g])
    dst = np.concatenate([i_g, u_idx.astype(np.int64)])

    deg = np.bincount(src, minlength=N)  # symmetric edge list: in-deg == out-deg
    a = np.where(deg > 0, 1.0 / np.sqrt(np.maximum(deg, 1)), 0.0).astype(np.float32)
    w = (a[src] * a[dst]).astype(np.float32)

    order = np.argsort(dst, kind="stable")
    src_s = src[order].astype(np.int32)
    dst_s = dst[order]
    w_s = w[order]

    nblk_tot = NPAD // 128
    blk = (dst_s >> 7).astype(np.int64)
    nb = np.bincount(blk, minlength=nblk_tot)
    cpb = int(np.ceil(nb.max() / 128))

    starts = np.zeros(nblk_tot, np.int64)
    np.cumsum(nb[:-1], out=starts[1:])
    r = np.arange(len(dst_s)) - starts[blk]
    gchunk = blk * cpb + (r >> 7)
    slot = r & 127

    nchunks_tot = nblk_tot * cpb
    srcmat = np.zeros((nchunks_tot, 128), np.int32)
    wmat = np.zeros((nchunks_tot, 128), np.float32)
    dlocmat = np.zeros((nchunks_tot, 128), np.float32)
    srcmat[gchunk, slot] = src_s
    wmat[gchunk, slot] = w_s
    dlocmat[gchunk, slot] = (dst_s & 127).astype(np.float32)

    x0 = np.concatenate([np.asarray(u_emb), np.asarray(i_emb)], axis=0)
    x0_pad = np.zeros((NPAD, D), np.float32)
    x0_pad[:N] = x0
    table0 = x0_pad.astype(NP_BF16)
    iota = np.tile(np.arange(128, dtype=np.float32), (128, 1))

    g = GB * cpb  # chunks per gather group
    in_maps = []
    for c in range(NCORES):
        lo, hi = c * NB * cpb, (c + 1) * NB * cpb
        # [nG, 128, G]: element [gi, p, j] belongs to chunk gi*G+j, slot p
        csrc = np.ascontiguousarray(
            srcmat[lo:hi].reshape(NG, g, 128).transpose(0, 2, 1))
        cw = np.ascontiguousarray(
            wmat[lo:hi].reshape(NG, g, 128).transpose(0, 2, 1))
        cdst = np.ascontiguousarray(
            dlocmat[lo:hi].reshape(NG, g, 128).transpose(0, 2, 1))
        in_maps.append({
            "table0": table0,
            "csrc": csrc,
            "cw": cw,
            "cdst": cdst,
            "x0own": np.ascontiguousarray(x0_pad[c * R:(c + 1) * R]),
            "iota": iota,
        })
    return in_maps, cpb


def _build_program(cpb):
    g = GB * cpb
    nc = bacc.Bacc("TRN2", target_bir_lowering=False, debug=False,
                   num_devices=NCORES)

    table0 = nc.dram_tensor("table0", [NPAD, D], BF16, kind="ExternalInput").ap()
    csrc = nc.dram_tensor("csrc", [NG, 128, g], I32, kind="ExternalInput").ap()
    cw = nc.dram_tensor("cw", [NG, 128, g], F32, kind="ExternalInput").ap()
    cdst = nc.dram_tensor("cdst", [NG, 128, g], F32, kind="ExternalInput").ap()
    x0own = nc.dram_tensor("x0own", [R, D], F32, kind="ExternalInput").ap()
    iota_in = nc.dram_tensor("iota", [128, 128], F32, kind="ExternalInput").ap()
    out = nc.dram_tensor("out", [R, D], F32, kind="ExternalOutput").ap()

    x1own_bf = nc.dram_tensor("x1own_bf", [R, D], BF16).ap()
    table1 = nc.dram_tensor("table1", [NPAD, D], BF16, addr_space="Shared").ap()

    with tile.TileContext(nc) as tc:
        with (
            tc.tile_pool(name="persist", bufs=1) as persist,
            tc.tile_pool(name="meta", bufs=3) as meta,
            tc.tile_pool(name="gather", bufs=3) as gp,
            tc.tile_pool(name="oh", bufs=8) as ohp,
            tc.tile_pool(name="ev", bufs=3) as ev,
            tc.tile_pool(name="psum", bufs=8, space="PSUM") as pp,
        ):
            iota_t = persist.tile([128, 128], F32)
            nc.sync.dma_start(out=iota_t[:], in_=iota_in[:])
            # fp32 copy of this core's x1 shard, kept in SBUF between hops
            x1keep = persist.tile([128, NB * D], F32)

            def smooth(hop, table_ap):
                for gi in range(NG):
                    csrc_t = meta.tile([128, g], I32, tag="csrc")
                    cw_t = meta.tile([128, g], F32, tag="cw")
                    cdst_t = meta.tile([128, g], F32, tag="cdst")
                    nc.sync.dma_start(out=csrc_t[:], in_=csrc[gi])
                    nc.sync.dma_start(out=cw_t[:], in_=cw[gi])
                    nc.sync.dma_start(out=cdst_t[:], in_=cdst[gi])

                    gbuf = gp.tile([128, g * D], BF16, tag="gbuf")
                    # HW indirect DMA consumes one index per dest partition
                    # row, so gather 128 rows per call.
                    for j in range(g):
                        nc.gpsimd.indirect_dma_start(
                            out=gbuf[:, j * D:(j + 1) * D], out_offset=None,
                            in_=table_ap,
                            in_offset=IndirectOffsetOnAxis(
                                ap=csrc_t[:, j:j + 1], axis=0),
                        )

                    for jb in range(GB):
                        b = gi * GB + jb
                        psum = pp.tile([128, D], F32, tag="psum")
                        for k in range(cpb):
                            j = jb * cpb + k
                            oh = ohp.tile([128, 128], BF16, tag="oh")
                            nc.vector.tensor_scalar(
                                out=oh[:], in0=iota_t[:],
                                scalar1=cdst_t[:, j:j + 1],
                                scalar2=cw_t[:, j:j + 1],
                                op0=mybir.AluOpType.is_equal,
                                op1=mybir.AluOpType.mult,
                            )
                            nc.tensor.matmul(
                                out=psum[:], lhsT=oh[:],
                                rhs=gbuf[:, j * D:(j + 1) * D],
                                start=(k == 0), stop=(k == cpb - 1),
                            )
                        rows = slice(b * 128, (b + 1) * 128)
                        if hop == 0:
                            x1bf = ev.tile([128, D], BF16, tag="x1bf")
                            nc.scalar.activation(
                                out=x1bf[:], in_=psum[:],
                                func=mybir.ActivationFunctionType.Copy)
                            nc.sync.dma_start(out=x1own_bf[rows], in_=x1bf[:])
                            nc.scalar.activation(
                                out=x1keep[:, b * D:(b + 1) * D], in_=psum[:],
                                func=mybir.ActivationFunctionType.Copy)
                        else:
                            x0blk = ev.tile([128, D], F32, tag="x0blk")
                            nc.sync.dma_start(out=x0blk[:], in_=x0own[rows])
                            s = ev.tile([128, D], F32, tag="s")
                            nc.vector.tensor_tensor(
                                out=s[:], in0=x0blk[:],
                                in1=x1keep[:, b * D:(b + 1) * D],
                                op=mybir.AluOpType.add)
                            t = ev.tile([128, D], F32, tag="t")
                            nc.vector.tensor_scalar(
                                out=t[:], in0=s[:], scalar1=2.0, scalar2=None,
                                op0=mybir.AluOpType.mult)
                            u = ev.tile([128, D], F32, tag="u")
                            nc.vector.tensor_tensor(
                                out=u[:], in0=t[:], in1=psum[:],
                                op=mybir.AluOpType.add)
                            obuf = ev.tile([128, D], F32, tag="obuf")
                            nc.scalar.activation(
                                out=obuf[:], in_=u[:],
                                func=mybir.ActivationFunctionType.Copy,
                                scale=1.0 / 3.0)
                            nc.sync.dma_start(out=out[rows], in_=obuf[:])

            smooth(0, table0[:])
            nc.gpsimd.collective_compute(
                "AllGather", mybir.AluOpType.bypass,
                replica_groups=[list(range(NCORES))],
                ins=[x1own_bf[:]], outs=[table1[:]],
            )
            smooth(1, table1[:])

    nc.compile()
    return nc


def _get_program(cpb):
    if cpb not in _PROG_CACHE:
        _PROG_CACHE[cpb] = _build_program(cpb)
    return _PROG_CACHE[cpb]


def kernel(u_emb, i_emb, u_idx, i_idx):
    in_maps, cpb = _host_prep(u_emb, i_emb, u_idx, i_idx)
    nc = _get_program(cpb)
    res = run_bass_kernel_spmd(nc, in_maps, list(range(NCORES)))
    full = np.concatenate([res.results[c]["out"] for c in range(NCORES)], axis=0)
    return np.ascontiguousarray(full[:N]).astype(np.float32)



# revision 9
# speedup vs baseline: 3.1630x; 3.1630x over previous
"""LightGCN 2-hop smoothing on 8 Trainium2 NeuronCores.

Strategy (edge-sharded by destination, transfer-minimized):
  - Host: build symmetric directed edge list (2E = 2.5M messages), compute
    per-node scale s = deg^-1/2 (s=1 for isolated nodes), sort edges by
    destination, and pack them into fixed-size 128-edge chunks grouped by
    128-node destination blocks. Core c owns destination nodes
    [c*25088, (c+1)*25088).
  - The symmetric GCN weight w_e = s[src]*s[dst] factorizes, so no per-edge
    weights are shipped: the gather table holds s-prescaled rows
    (t0 = s .* x0 in bf16) and each scatter-add output row is postscaled by
    s[dst] on device. Per-edge metadata is just src index (int32) and
    dst slot within the 128-block (bf16; 255 marks padding slots).
  - Each core receives only its 1/8 table shard; the replicated gather table
    is built on device with an AllGather over NeuronLink.
  - Device, per smoothing hop: gather source rows with indirect DMA, build a
    one-hot selection matrix per 128-edge chunk on the DVE
    (out[p,f] = (f == dstloc[p])), and matmul-accumulate the chunk's
    messages into a PSUM tile per destination block.
  - x0 is recovered on device from the scaled bf16 shard (x0 = sinv .* t0),
    and the final out = (2*x0 + 2*x1 + x2)/3 is emitted in bf16.
"""

import numpy as np

import concourse.bass as bass
import concourse.bacc as bacc
import concourse.mybir as mybir
import concourse.tile as tile
from concourse.bass import IndirectOffsetOnAxis
from concourse.bass_utils import run_bass_kernel_spmd

NU = 100000          # num users
NI = 100000          # num items
N = NU + NI          # real nodes
D = 64               # embedding dim
NCORES = 8
R = 25088            # padded rows per core (196 blocks of 128)
NPAD = R * NCORES    # 200704 padded node table rows
NB = 196             # destination blocks per core
GB = 4               # blocks per gather group
NG = NB // GB        # gather groups per core

F32 = mybir.dt.float32
BF16 = mybir.dt.bfloat16
I32 = mybir.dt.int32
NP_BF16 = mybir.dt.np(mybir.dt.bfloat16)

_PROG_CACHE = {}


def _host_prep(u_emb, i_emb, u_idx, i_idx):
    i_g = i_idx.astype(np.int64) + NU
    src = np.concatenate([u_idx.astype(np.int64), i_g])
    dst = np.concatenate([i_g, u_idx.astype(np.int64)])

    deg = np.bincount(src, minlength=NPAD)  # symmetric edges: in-deg == out-deg
    # s = deg^-1/2 where deg>0 else 1; w_e = s[src]*s[dst] (isolated nodes
    # never appear in any edge, so s=1 there is never used as a weight and
    # keeps x0 = sinv*(s*x0) exact for them).
    s = np.where(deg > 0, 1.0 / np.sqrt(np.maximum(deg, 1)), 1.0).astype(np.float32)
    sinv = (1.0 / s).astype(np.float32)

    order = np.argsort(dst, kind="stable")
    src_s = src[order].astype(np.int32)
    dst_s = dst[order]

    nblk_tot = NPAD // 128
    blk = (dst_s >> 7).astype(np.int64)
    nb = np.bincount(blk, minlength=nblk_tot)
    cpb = int(np.ceil(nb.max() / 128))

    starts = np.zeros(nblk_tot, np.int64)
    np.cumsum(nb[:-1], out=starts[1:])
    r = np.arange(len(dst_s)) - starts[blk]
    gchunk = blk * cpb + (r >> 7)
    slot = r & 127

    nchunks_tot = nblk_tot * cpb
    srcmat = np.zeros((nchunks_tot, 128), np.int32)
    dlocmat = np.full((nchunks_tot, 128), 255.0, np.float32)  # 255 = padding
    srcmat[gchunk, slot] = src_s
    dlocmat[gchunk, slot] = (dst_s & 127).astype(np.float32)
    dlocmat = dlocmat.astype(NP_BF16)

    x0 = np.concatenate([np.asarray(u_emb), np.asarray(i_emb)], axis=0)
    t0_pad = np.zeros((NPAD, D), np.float32)
    t0_pad[:N] = x0 * s[:N, None]
    t0_pad = t0_pad.astype(NP_BF16)
    iota = np.tile(np.arange(128, dtype=np.float32), (128, 1))

    g = GB * cpb  # chunks per gather group
    in_maps = []
    for c in range(NCORES):
        lo, hi = c * NB * cpb, (c + 1) * NB * cpb
        # [nG, 128, G]: element [gi, p, j] belongs to chunk gi*G+j, slot p
        csrc = np.ascontiguousarray(
            srcmat[lo:hi].reshape(NG, g, 128).transpose(0, 2, 1))
        cdst = np.ascontiguousarray(
            dlocmat[lo:hi].reshape(NG, g, 128).transpose(0, 2, 1))
        rows = slice(c * R, (c + 1) * R)
        in_maps.append({
            "t0": np.ascontiguousarray(t0_pad[rows]),
            "csrc": csrc,
            "cdst": cdst,
            "s2d": np.ascontiguousarray(s[rows].reshape(NB, 128).T),
            "sinv2d": np.ascontiguousarray(sinv[rows].reshape(NB, 128).T),
            "iota": iota,
        })
    return in_maps, cpb


def _build_program(cpb):
    g = GB * cpb
    nc = bacc.Bacc("TRN2", target_bir_lowering=False, debug=False,
                   num_devices=NCORES)

    t0_in = nc.dram_tensor("t0", [R, D], BF16, kind="ExternalInput").ap()
    csrc = nc.dram_tensor("csrc", [NG, 128, g], I32, kind="ExternalInput").ap()
    cdst = nc.dram_tensor("cdst", [NG, 128, g], BF16, kind="ExternalInput").ap()
    s2d = nc.dram_tensor("s2d", [128, NB], F32, kind="ExternalInput").ap()
    sinv2d = nc.dram_tensor("sinv2d", [128, NB], F32, kind="ExternalInput").ap()
    iota_in = nc.dram_tensor("iota", [128, 128], F32, kind="ExternalInput").ap()
    out = nc.dram_tensor("out", [R, D], BF16, kind="ExternalOutput").ap()

    t0i = nc.dram_tensor("t0i", [R, D], BF16).ap()
    x1own_bf = nc.dram_tensor("x1own_bf", [R, D], BF16).ap()
    table0 = nc.dram_tensor("table0", [NPAD, D], BF16, addr_space="Shared").ap()
    table1 = nc.dram_tensor("table1", [NPAD, D], BF16, addr_space="Shared").ap()

    with tile.TileContext(nc) as tc:
        with (
            tc.tile_pool(name="persist", bufs=1) as persist,
            tc.tile_pool(name="meta", bufs=3) as meta,
            tc.tile_pool(name="gather", bufs=3) as gp,
            tc.tile_pool(name="oh", bufs=8) as ohp,
            tc.tile_pool(name="ev", bufs=3) as ev,
            tc.tile_pool(name="psum", bufs=8, space="PSUM") as pp,
        ):
            iota_t = persist.tile([128, 128], F32)
            nc.sync.dma_start(out=iota_t[:], in_=iota_in[:])
            s_t = persist.tile([128, NB], F32)
            nc.sync.dma_start(out=s_t[:], in_=s2d[:])
            sinv_t = persist.tile([128, NB], F32)
            nc.sync.dma_start(out=sinv_t[:], in_=sinv2d[:])
            # fp32 copy of this core's x1 shard, kept in SBUF between hops
            x1keep = persist.tile([128, NB * D], F32)

            # replicate the scaled-x0 shards into the gather table
            # (collectives cannot read IO tensors; stage through t0i)
            nc.sync.dma_start(out=t0i[:], in_=t0_in[:])
            nc.gpsimd.collective_compute(
                "AllGather", mybir.AluOpType.bypass,
                replica_groups=[list(range(NCORES))],
                ins=[t0i[:]], outs=[table0[:]],
            )

            def smooth(hop, table_ap):
                for gi in range(NG):
                    csrc_t = meta.tile([128, g], I32, tag="csrc")
                    cdst_t = meta.tile([128, g], BF16, tag="cdst")
                    nc.sync.dma_start(out=csrc_t[:], in_=csrc[gi])
                    nc.sync.dma_start(out=cdst_t[:], in_=cdst[gi])
                    cdstf = meta.tile([128, g], F32, tag="cdstf")
                    nc.scalar.activation(
                        out=cdstf[:], in_=cdst_t[:],
                        func=mybir.ActivationFunctionType.Copy)

                    gbuf = gp.tile([128, g * D], BF16, tag="gbuf")
                    # HW indirect DMA consumes one index per dest partition
                    # row, so gather 128 rows per call.
                    for j in range(g):
                        nc.gpsimd.indirect_dma_start(
                            out=gbuf[:, j * D:(j + 1) * D], out_offset=None,
                            in_=table_ap,
                            in_offset=IndirectOffsetOnAxis(
                                ap=csrc_t[:, j:j + 1], axis=0),
                        )

                    for jb in range(GB):
                        b = gi * GB + jb
                        psum = pp.tile([128, D], F32, tag="psum")
                        for k in range(cpb):
                            j = jb * cpb + k
                            oh = ohp.tile([128, 128], BF16, tag="oh")
                            nc.vector.tensor_scalar(
                                out=oh[:], in0=iota_t[:],
                                scalar1=cdstf[:, j:j + 1], scalar2=None,
                                op0=mybir.AluOpType.is_equal,
                            )
                            nc.tensor.matmul(
                                out=psum[:], lhsT=oh[:],
                                rhs=gbuf[:, j * D:(j + 1) * D],
                                start=(k == 0), stop=(k == cpb - 1),
                            )
                        rows = slice(b * 128, (b + 1) * 128)
                        if hop == 0:
                            # x1 = s * psum (keep f32); table1 row = s * x1
                            nc.vector.tensor_scalar(
                                out=x1keep[:, b * D:(b + 1) * D], in0=psum[:],
                                scalar1=s_t[:, b:b + 1], scalar2=None,
                                op0=mybir.AluOpType.mult)
                            x1s = ev.tile([128, D], BF16, tag="x1s")
                            nc.scalar.activation(
                                out=x1s[:], in_=x1keep[:, b * D:(b + 1) * D],
                                func=mybir.ActivationFunctionType.Copy,
                                scale=s_t[:, b:b + 1])
                            nc.sync.dma_start(out=x1own_bf[rows], in_=x1s[:])
                        else:
                            t0blk = ev.tile([128, D], BF16, tag="t0blk")
                            nc.sync.dma_start(out=t0blk[:], in_=t0_in[rows])
                            x0f = ev.tile([128, D], F32, tag="x0f")
                            nc.scalar.activation(
                                out=x0f[:], in_=t0blk[:],
                                func=mybir.ActivationFunctionType.Copy,
                                scale=sinv_t[:, b:b + 1])
                            a01 = ev.tile([128, D], F32, tag="a01")
                            nc.vector.tensor_tensor(
                                out=a01[:], in0=x0f[:],
                                in1=x1keep[:, b * D:(b + 1) * D],
                                op=mybir.AluOpType.add)
                            x2t = ev.tile([128, D], F32, tag="x2t")
                            nc.scalar.activation(
                                out=x2t[:], in_=psum[:],
                                func=mybir.ActivationFunctionType.Copy,
                                scale=s_t[:, b:b + 1])
                            a2 = ev.tile([128, D], F32, tag="a2")
                            nc.vector.tensor_scalar(
                                out=a2[:], in0=a01[:], scalar1=2.0,
                                scalar2=None, op0=mybir.AluOpType.mult)
                            u = ev.tile([128, D], F32, tag="u")
                            nc.vector.tensor_tensor(
                                out=u[:], in0=a2[:], in1=x2t[:],
                                op=mybir.AluOpType.add)
                            obuf = ev.tile([128, D], BF16, tag="obuf")
                            nc.scalar.activation(
                                out=obuf[:], in_=u[:],
                                func=mybir.ActivationFunctionType.Copy,
                                scale=1.0 / 3.0)
                            nc.sync.dma_start(out=out[rows], in_=obuf[:])

            smooth(0, table0[:])
            nc.gpsimd.collective_compute(
                "AllGather", mybir.AluOpType.bypass,
                replica_groups=[list(range(NCORES))],
                ins=[x1own_bf[:]], outs=[table1[:]],
            )
            smooth(1, table1[:])

    nc.compile()
    return nc


def _get_program(cpb):
    if cpb not in _PROG_CACHE:
        _PROG_CACHE[cpb] = _build_program(cpb)
    return _PROG_CACHE[cpb]


def kernel(u_emb, i_emb, u_idx, i_idx):
    in_maps, cpb = _host_prep(u_emb, i_emb, u_idx, i_idx)
    nc = _get_program(cpb)
    res = run_bass_kernel_spmd(nc, in_maps, list(range(NCORES)))
    full = np.concatenate([res.results[c]["out"] for c in range(NCORES)], axis=0)
    return np.ascontiguousarray(full[:N]).astype(np.float32)
